# revision 1
# baseline (speedup 1.0000x reference)
"""CRF loss (forward-algorithm partition + gold-path score) on 8 Trainium2 cores.

Data-parallel over batch (256/8 = 32 per core). Two probability-space scans
run per core, both as PE matmuls over [tag=128 part, batch=32 free] states:

  X scan (partition):  X <- (E'^T X) * w_s,   E'  = exp(trans) * 2^-9
  g scan (gold path):  g <- (E''^T g) * w_s * onehot(tag_s),  E'' = exp(trans)

The masked gold scan keeps exactly the gold path's probability, so its
accumulated log-normalizer is emit_score + trans_score + boundary terms, and
loss_b = partition_b - gold_b with no gather ops anywhere. Both scans renorm
every 32 steps by their column sums (ones-matmul + reciprocal + multiply),
deferring all Ln's to one ACT pass at the end. One-hot masks are built per
32-step chunk from a host-relayouted tagsQ via one stride-0-broadcast DVE
compare + 8 PE transposes, then fused into wO = onehot * w during PSUM
evacuation. Emissions are host-pre-permuted to [S, T, Bc] so each chunk is
one contiguous DMA and one ACT Exp (fp32 in, bf16 out). Scans run in bf16
(fp32 PSUM accumulate); the scalar loss only needs ~1e-4 relative accuracy.
"""

import sys

import numpy as np

sys.path.insert(0, "/opt/trn_rl_repo")

import concourse.bacc as bacc_mod
import concourse.bass as bass
import concourse.mybir as mybir
import concourse.tile as tile
from concourse.bass_utils import run_bass_kernel_spmd

B, S, T = 256, 1024, 128
NCORES = 8
Bc = B // NCORES  # 32
START, END = T - 2, T - 1  # 126, 127
K = 32          # renorm period (steps)
CHUNK = 32      # scan steps per emissions DMA/exp chunk
NSTEPS = S - 1  # X scan: s = 1..1023 (emissions[:, 0, :] never enters partition)
PRE_BITS = 9.0  # E' prescale 2^-9 keeps X shrinking ~0.68x/step on average
BIAS0 = float(-PRE_BITS * np.log(2.0))
RENORM_STEPS = [s for s in range(1, NSTEPS + 1) if s % K == 0 and s != NSTEPS]
NR = len(RENORM_STEPS)
F32 = mybir.dt.float32
BF16 = mybir.dt.bfloat16
I32 = mybir.dt.int32


def _build_kernel(debug: bool = False) -> bass.Bass:
    nc = bacc_mod.Bacc()
    emT = nc.dram_tensor("emT", [S, T, Bc], F32, kind="ExternalInput")
    tagsQ_d = nc.dram_tensor("tagsQ", [T, S // 4], I32, kind="ExternalInput")
    trans_d = nc.dram_tensor("trans", [T, T], F32, kind="ExternalInput")
    partX_out = nc.dram_tensor("partX", [1, Bc], F32, kind="ExternalOutput")
    partG_out = nc.dram_tensor("partG", [1, Bc], F32, kind="ExternalOutput")
    if debug:
        dbg_xf = nc.dram_tensor("dbg_xf", [T, Bc], F32, kind="ExternalOutput")
        dbg_gf = nc.dram_tensor("dbg_gf", [T, Bc], F32, kind="ExternalOutput")
        dbg_zx = nc.dram_tensor("dbg_zx", [1, max(NR, 1) * Bc], F32, kind="ExternalOutput")
        dbg_zg = nc.dram_tensor("dbg_zg", [1, max(NR, 1) * Bc], F32, kind="ExternalOutput")
        dbg_wo = nc.dram_tensor("dbg_wo", [T, CHUNK * Bc], F32, kind="ExternalOutput")

    Exp = mybir.ActivationFunctionType.Exp
    Copy = mybir.ActivationFunctionType.Copy
    Ln = mybir.ActivationFunctionType.Ln
    AX = mybir.AxisListType.X
    Alu = mybir.AluOpType

    with tile.TileContext(nc) as tc:
        with (
            tc.tile_pool(name="constp", bufs=1) as constp,
            tc.tile_pool(name="chunkp", bufs=3) as chunkp,
            tc.tile_pool(name="statep", bufs=4) as statep,
            tc.tile_pool(name="miscp", bufs=1) as miscp,
            tc.tile_pool(name="psump", bufs=2, space="PSUM") as psump,
            tc.tile_pool(name="psumo", bufs=2, space="PSUM") as psumo,
        ):
            # ---- constants ----
            trans_t = constp.tile([T, T], F32)
            nc.sync.dma_start(out=trans_t[:], in_=trans_d[:, :])
            bias0_t = constp.tile([T, 1], F32)
            nc.vector.memset(bias0_t[:], BIAS0)
            zero_t = constp.tile([T, 1], F32)
            nc.vector.memset(zero_t[:], 0.0)
            Ep = constp.tile([T, T], BF16)      # exp(trans) * 2^-9  (X scan)
            nc.scalar.activation(Ep[:], trans_t[:], Exp, bias=bias0_t[:])
            Epp = constp.tile([T, T], BF16)     # exp(trans)         (gold scan)
            nc.scalar.activation(Epp[:], trans_t[:], Exp, bias=zero_t[:])
            ones_t = constp.tile([T, T], BF16)
            nc.vector.memset(ones_t[:], 1.0)
            Efin = constp.tile([T, 1], BF16)
            nc.scalar.activation(Efin[:], trans_t[:, END : END + 1], Exp, bias=zero_t[:])

            # partition iota, free-dim iota, identity (for PE transpose)
            pid = constp.tile([T, 1], I32)
            nc.gpsimd.iota(pid[:], pattern=[[0, 1]], base=0, channel_multiplier=1)
            fid = constp.tile([T, T], I32)
            nc.gpsimd.iota(fid[:], pattern=[[1, T]], base=0, channel_multiplier=0)
            ident = constp.tile([T, T], BF16)
            nc.vector.tensor_tensor(
                out=ident[:], in0=pid[:].to_broadcast([T, T]), in1=fid[:], op=Alu.is_equal
            )

            tagsQ = constp.tile([T, S // 4], I32)
            nc.sync.dma_start(out=tagsQ[:], in_=tagsQ_d[:, :])

            # ---- scan state ----
            zvalsX = miscp.tile([1, max(NR, 1) * Bc], F32)
            zvalsG = miscp.tile([1, max(NR, 1) * Bc], F32)

            X = statep.tile([T, Bc], BF16, tag="X")
            nc.vector.tensor_scalar(
                out=X[:], in0=pid[:].to_broadcast([T, Bc]),
                scalar1=START, scalar2=None, op0=Alu.is_equal,
            )
            g = statep.tile([T, Bc], BF16, tag="g")
            nc.vector.tensor_scalar(
                out=g[:], in0=pid[:].to_broadcast([T, Bc]),
                scalar1=START, scalar2=None, op0=Alu.is_equal,
            )

            ren = 0
            for c in range(S // CHUNK):
                # emissions chunk: DMA fp32 [T, (s, b)] then w = exp() in bf16
                raw = chunkp.tile([T, CHUNK * Bc], F32, tag="raw")
                src = emT[c * CHUNK : (c + 1) * CHUNK, :, :].rearrange("s t b -> t s b")
                nc.sync.dma_start(
                    out=raw[:].rearrange("t (s b) -> t s b", s=CHUNK), in_=src
                )
                wch = chunkp.tile([T, CHUNK * Bc], BF16, tag="w")
                nc.scalar.activation(wch[:], raw[:], Exp, bias=zero_t[:])

                # one-hot masks for this chunk: maskQ[(sm,b), (sql, j)] then
                # 8 PE transposes -> O blocks [j, (sm, b)] -> wO = O * w
                mq = chunkp.tile([T, 8 * T], BF16, tag="mq")
                tq = tagsQ[:, c * 8 : (c + 1) * 8]
                nc.vector.tensor_tensor(
                    out=mq[:].rearrange("p (q j) -> p q j", q=8),
                    in0=fid[:, 0:T].rearrange("p (q j) -> p q j", q=1).to_broadcast([T, 8, T]),
                    in1=tq.rearrange("p (q j) -> p q j", j=1).to_broadcast([T, 8, T]),
                    op=Alu.is_equal,
                )
                wO = chunkp.tile([T, CHUNK * Bc], BF16, tag="wO")
                for sql in range(8):
                    op = psumo.tile([T, T], BF16, tag="op")
                    nc.tensor.transpose(
                        out=op[:], in_=mq[:, sql * T : (sql + 1) * T], identity=ident[:]
                    )
                    ob = chunkp.tile([T, T], BF16, tag="ob", bufs=2)
                    nc.scalar.activation(ob[:], op[:], Copy)
                    cols = slice(4 * sql * Bc, (4 * sql + 4) * Bc)
                    nc.vector.tensor_mul(out=wO[:, cols], in0=wch[:, cols], in1=ob[:])
                if debug and c == 0:
                    nc.gpsimd.dma_start(out=dbg_wo[:, :], in_=wO[:])

                for sl in range(CHUNK):
                    s = c * CHUNK + sl
                    wcols = slice(sl * Bc, (sl + 1) * Bc)
                    # gold scan: steps s = 0..1023
                    r = psump.tile([T, Bc], F32, tag="r")
                    nc.tensor.matmul(out=r[:], lhsT=Epp[:], rhs=g[:], start=True, stop=True)
                    gn = statep.tile([T, Bc], BF16, tag="g")
                    nc.vector.tensor_mul(out=gn[:], in0=wO[:, wcols], in1=r[:])
                    g = gn
                    # partition scan: steps s = 1..1023
                    if 1 <= s <= NSTEPS:
                        q = psump.tile([T, Bc], F32, tag="q")
                        nc.tensor.matmul(out=q[:], lhsT=Ep[:], rhs=X[:], start=True, stop=True)
                        Xn = statep.tile([T, Bc], BF16, tag="X")
                        nc.vector.tensor_mul(out=Xn[:], in0=wch[:, wcols], in1=q[:])
                        X = Xn
                    if s in RENORM_STEPS:
                        for st, zv, tagc in ((X, zvalsX, "X"), (g, zvalsG, "g")):
                            zb = psump.tile([T, Bc], F32, tag="zb", bufs=1)
                            nc.tensor.matmul(
                                out=zb[:], lhsT=ones_t[:], rhs=st[:], start=True, stop=True
                            )
                            zrec = statep.tile([T, Bc], F32, tag="zrec")
                            nc.vector.reciprocal(out=zrec[:], in_=zb[:])
                            stn = statep.tile([T, Bc], BF16, tag=tagc)
                            nc.vector.tensor_mul(out=stn[:], in0=st[:], in1=zrec[:])
                            nc.vector.tensor_copy(
                                out=zv[:, ren * Bc : (ren + 1) * Bc], in_=zb[0:1, :]
                            )
                            if tagc == "X":
                                X = stn
                            else:
                                g = stn
                        ren += 1

            # ---- final: partX = ln(sum_j X) (+ NEG on host, from reference's
            # all -10000 transitions[end] row); partG = ln(Efin . g) ----
            for st, zv, out_d, lhs in (
                (X, zvalsX, partX_out, ones_t[:, 0:1]),
                (g, zvalsG, partG_out, Efin[:]),
            ):
                fin = psump.tile([1, Bc], F32, tag="zb", bufs=1)
                nc.tensor.matmul(out=fin[:], lhsT=lhs, rhs=st[:], start=True, stop=True)
                lnfin = miscp.tile([1, Bc], F32)
                nc.scalar.activation(lnfin[:], fin[:], Ln, bias=zero_t[0:1, :])
                lnz = miscp.tile([1, max(NR, 1) * Bc], F32)
                nc.scalar.activation(
                    lnz[:, 0 : NR * Bc], zv[:, 0 : NR * Bc], Ln, bias=zero_t[0:1, :]
                )
                zsum = miscp.tile([1, Bc], F32)
                nc.vector.reduce_sum(
                    out=zsum[:],
                    in_=lnz[:, 0 : NR * Bc].rearrange("p (r b) -> p b r", b=Bc),
                    axis=AX,
                )
                part = miscp.tile([1, Bc], F32)
                nc.vector.tensor_add(out=part[:], in0=lnfin[:], in1=zsum[:])
                nc.sync.dma_start(out=out_d[:, :], in_=part[:])
            if debug:
                nc.gpsimd.dma_start(out=dbg_xf[:, :], in_=X[:])
                nc.gpsimd.dma_start(out=dbg_gf[:, :], in_=g[:])
                nc.sync.dma_start(out=dbg_zx[:, :], in_=zvalsX[:])
                nc.sync.dma_start(out=dbg_zg[:, :], in_=zvalsG[:])

    nc.compile()
    return nc


def make_tagsQ(tags_core: np.ndarray) -> np.ndarray:
    """[Bc, S] int32 -> [128, S//4] with tagsQ[sm*32+b, sq] = tags[b, 4*sq+sm]."""
    t = tags_core.reshape(Bc, S // 4, 4)            # [b, sq, sm]
    return np.ascontiguousarray(t.transpose(2, 0, 1).reshape(4 * Bc, S // 4)).astype(np.int32)


_NC_CACHE: list = []


def kernel(emissions: np.ndarray, tags: np.ndarray, transitions: np.ndarray) -> np.ndarray:
    emissions = np.ascontiguousarray(np.asarray(emissions, dtype=np.float32))
    tags_np = np.asarray(tags).astype(np.int32)
    transitions = np.ascontiguousarray(np.asarray(transitions, dtype=np.float32))

    if not _NC_CACHE:
        _NC_CACHE.append(_build_kernel())
    nc = _NC_CACHE[0]

    in_maps = []
    for c in range(NCORES):
        sl = slice(c * Bc, (c + 1) * Bc)
        in_maps.append(
            {
                "emT": np.ascontiguousarray(emissions[sl].transpose(1, 2, 0)),
                "tagsQ": make_tagsQ(tags_np[sl]),
                "trans": transitions,
            }
        )

    kernel._last_in_maps = in_maps
    results = run_bass_kernel_spmd(nc, in_maps, core_ids=list(range(NCORES))).results

    constX = np.float64(NSTEPS * PRE_BITS * np.log(2.0))
    total = np.float64(0.0)
    for c in range(NCORES):
        r = results[c]
        px = r["partX"].reshape(-1).astype(np.float64) + constX - 10000.0
        pg = r["partG"].reshape(-1).astype(np.float64)
        total += (px - pg).sum()

    return np.array(total / B, dtype=np.float32)

